# revision 2
# baseline (speedup 1.0000x reference)
"""Trainium2 Bass kernel for nn_MemTransformerLM (Transformer-XL layer).

Sharding (8 cores): batch (4) x head-half (2). Every core runs an identical
program over batch b = c//2 and heads [hh*8, hh*8+8), hh = c%2, for all 1024
queries. After o_proj a 2-rank ReduceScatter over core pairs (2b, 2b+1)
splits tokens for the FFN: even core keeps tokens [0,512), odd [512,1024).

Attention rel-shift: BD[i,j] = BD_raw[i, j-i+Q-1] is applied with a
"diagonal" SBUF->SBUF DMA (flat access pattern [[W-1,128],[1,N]]) that
accumulates the shifted BD window into the AC scores. The causal mask is
baked in by memsetting the out-of-range tail of each BD window to -30
before the shift, so exp() zeroes masked lanes without a mask pass.
"""

import contextlib
import math

import numpy as np

import concourse.bass as bass
import concourse.bacc as bacc
import concourse.mybir as mybir
import concourse.tile as tile
from concourse.masks import make_identity

F32 = mybir.dt.float32
BF16 = mybir.dt.bfloat16
AF = mybir.ActivationFunctionType
ALU = mybir.AluOpType


class Cfg:
    D = 1024      # model dim
    NHC = 8       # heads per core
    DH = 64       # head dim
    KL = 2048     # key length
    Q = 1024      # query length
    DI = 4096     # ffn inner
    LN_EPS = 1e-5
    N_CORES = 8

    HD = property(lambda s: s.NHC * s.DH)       # head dims per core
    SCALE = property(lambda s: 1.0 / (s.DH ** 0.5))
    M = property(lambda s: s.KL - s.Q)          # mem length
    NS = property(lambda s: s.Q // 128)         # q tiles
    NJT = property(lambda s: s.KL // 128)       # key tiles
    DPT = property(lambda s: s.D // 128)
    HPT = property(lambda s: s.HD // 128)
    NTT = property(lambda s: s.KL // 128)
    WB = property(lambda s: s.KL + 128)         # BD window buffer width
    TOKF = property(lambda s: s.Q // 2)         # ffn tokens per core

    def jmax(self, s):
        return min(self.KL, 128 * (s + 1) + self.M)

    def jcomp(self, s):
        # 128-aligned causal bound (jmax is already 128-aligned)
        return self.jmax(s)

    def wstart(self, s):
        return self.Q - 128 * (s + 1)

    def wreal(self, s):
        return min(self.jcomp(s) + 128, self.KL - self.wstart(s))


def ts(i, n):
    return slice(i * n, (i + 1) * n)


def chunks(total, sz=512):
    return [(lo, min(total, lo + sz)) for lo in range(0, total, sz)]


def build_kernel(c: Cfg = None, collective=True):
    c = c or Cfg()
    nc = bacc.Bacc("TRN2", target_bir_lowering=False)

    io = {}
    def din(name, shape):
        io[name] = nc.dram_tensor(name, shape, F32, kind="ExternalInput")
    din("xw", [c.KL, c.D])
    din("r_in", [c.KL, c.D])
    din("qkvw", [c.D, 3 * c.HD])
    din("rnetw", [c.D, c.HD])
    din("oww", [c.HD, c.D])
    din("rwb", [1, c.HD])
    din("rrb", [1, c.HD])
    din("ln1g", [1, c.D]); din("ln1b", [1, c.D])
    din("ln2g", [1, c.D]); din("ln2b", [1, c.D])
    din("ffw1", [c.D, c.DI]); din("ffb1", [1, c.DI])
    din("ffw2", [c.DI, c.D]); din("ffb2", [1, c.D])
    din("wres", [c.TOKF, c.D])
    io["out"] = nc.dram_tensor("out", [c.TOKF, c.D], F32, kind="ExternalOutput")
    io["rs_bin"] = nc.dram_tensor("rs_bin", [c.Q, c.D], F32)
    io["rs_bout"] = nc.dram_tensor("rs_bout", [c.TOKF, c.D], F32)

    with tile.TileContext(nc) as tc:
        _body(tc, nc, c, io, collective=collective)
    nc.finalize()
    return nc


def _qslice(buf, c, hp, hr, s):
    """[64,128] lhsT slice for head (hp, hr) and q-tile s of a [128, HPT*Q] buf."""
    return buf[hr:hr + 64, hp * c.Q + s * 128: hp * c.Q + (s + 1) * 128]


def _body(tc, nc, c, io, collective=True):
    ctx = contextlib.ExitStack()
    rg = [[i, i + 1] for i in range(0, c.N_CORES, 2)]
    with ctx:
        small = ctx.enter_context(tc.tile_pool(name="small", bufs=2))
        psA = ctx.enter_context(tc.tile_pool(name="psA", bufs=3, space="PSUM"))
        psB = ctx.enter_context(tc.tile_pool(name="psB", bufs=3, space="PSUM"))
        psV = ctx.enter_context(tc.tile_pool(name="psV", bufs=2, space="PSUM"))

        def ps_a():
            return psA.tile([128, 512], F32, tag="a", name="psa")

        def ps_b():
            return psB.tile([128, 512], F32, tag="b", name="psb")

        keep = ctx.enter_context(tc.tile_pool(name="keep", bufs=1))
        ident = keep.tile([128, 128], BF16, tag="identb")
        make_identity(nc, ident)
        identf = keep.tile([128, 128], F32, tag="identf")
        make_identity(nc, identf)

        # ============ phase A/B: streamed loads + projections ============
        # Loads are issued load-first in quarter batches (one DMA per 512
        # DRAM rows) so transposes never head-block loads on the sync queue;
        # casts chase loads on DVE, transposes chase casts, projection
        # matmuls chase transposed quarters.
        atp = tc.alloc_tile_pool(name="atp", bufs=1)
        attk = tc.alloc_tile_pool(name="attk", bufs=1)
        rTp = attk.tile([128, c.HPT * c.WB], BF16, tag="rTp")
        kT = attk.tile([128, c.HPT * c.KL], BF16, tag="kT")
        VW = c.NHC * 65
        vb = attk.tile([128, c.NTT * VW], BF16, tag="vb")
        rwq = attk.tile([128, c.HPT * c.Q], BF16, tag="rwq")
        rrq = attk.tile([128, c.HPT * c.Q], BF16, tag="rrq")
        attnT = atp.tile([128, c.HPT * c.Q], BF16, tag="attnT")
        ow_t = []

        # biases (scalar queue; tiny)
        rwb_s = keep.tile([128, c.HPT], F32, tag="rwb")
        rrb_s = keep.tile([128, c.HPT], F32, tag="rrb")
        nc.scalar.dma_start(out=rwb_s[:], in_=bass.AP(
            tensor=io["rwb"].ap().tensor, offset=0, ap=[[1, 128], [128, c.HPT]]))
        nc.scalar.dma_start(out=rrb_s[:], in_=bass.AP(
            tensor=io["rrb"].ap().tensor, offset=0, ap=[[1, 128], [128, c.HPT]]))

        def load_rows(src, row0, nrows, width, pool, tag="ldq", bufs=None):
            """DRAM [*, width] rows [row0, row0+nrows) -> [128, n*width]."""
            n = nrows // 128
            t = pool.tile([128, n * width], F32, tag=tag, bufs=bufs)
            ap = bass.AP(tensor=src.ap().tensor, offset=row0 * width,
                         ap=[[width, 128], [128 * width, n], [1, width]])
            nc.scalar.dma_start(out=t[:], in_=ap)
            return t

        def transpose_quarter(qtile, dst, ldc):
            """cast+transpose a [128, 4*D] f32 quarter -> [128, DPT*512] bf16."""
            for j in range(4):
                bft = ldc.tile([128, c.D], BF16, tag="castA")
                nc.vector.tensor_copy(out=bft[:], in_=qtile[:, j * c.D:(j + 1) * c.D])
                dstap = bass.AP(
                    tensor=dst.tensor, offset=dst.offset + j * 128,
                    ap=[[c.DPT * 512, 128], [512, c.DPT], [1, 128]])
                nc.sync.dma_start(out=dstap, in_=bft[:], transpose=True)

        with tc.tile_pool(name="ldq", bufs=2) as ldq, \
             tc.tile_pool(name="tq", bufs=2) as tqp, \
             tc.tile_pool(name="ldc", bufs=3) as ldc, \
             tc.tile_pool(name="wpool", bufs=1) as wpool:
            # --- R: load, transpose, project; quarter-local transposed tiles
            rnw = load_rows(io["rnetw"], 0, c.D, c.HD, ldq, tag="ldw", bufs=1)
            rq = [load_rows(io["r_in"], qd * 512, 512, c.D, ldq) for qd in range(2)]
            wr_t = []
            for p in range(c.DPT):
                bt = wpool.tile([128, c.HD], BF16, tag="wr_%d" % p)
                nc.vector.tensor_copy(out=bt[:], in_=rnw[:, p * c.HD:(p + 1) * c.HD])
                wr_t.append(bt)
            for m in range(c.HPT):
                nc.gpsimd.memset(rTp[:, m * c.WB + c.KL:(m + 1) * c.WB], 0.0)
            for qd in range(4):
                tq = tqp.tile([128, c.DPT * 512], BF16, tag="tq")
                transpose_quarter(rq[qd], tq, ldc)
                if qd + 2 < 4:
                    rq.append(load_rows(io["r_in"], (qd + 2) * 512, 512, c.D, ldq))
                lo = qd * 512
                for m in range(c.HPT):
                    ps = ps_a()
                    for k in range(c.DPT):
                        nc.tensor.matmul(
                            ps[:], wr_t[k][:, ts(m, 128)], tq[:, ts(k, 512)],
                            start=(k == 0), stop=(k == c.DPT - 1))
                    nc.scalar.activation(
                        out=rTp[:, m * c.WB + lo: m * c.WB + lo + 512],
                        in_=ps[:], func=AF.Copy)
            # --- qkv weights (single ldw buffer; halves sequential)
            wq_t, wk_t, wv_t = [], [], []
            for p2 in range(2):
                qkw = load_rows(io["qkvw"], p2 * 512, 512, 3 * c.HD, ldq,
                                tag="ldw", bufs=1)
                for pp in range(4):
                    p = p2 * 4 + pp
                    src = qkw[:, pp * 3 * c.HD:]
                    for lst, j, tag in ((wq_t, 0, "wq"), (wk_t, 1, "wk"), (wv_t, 2, "wv")):
                        bt = wpool.tile([128, c.HD], BF16, tag="%s_%d" % (tag, p))
                        nc.vector.tensor_copy(
                            out=bt[:], in_=src[:, j * c.HD:(j + 1) * c.HD])
                        lst.append(bt)
            # --- X: load, transpose, K/V/Q per quarter
            xq = [load_rows(io["xw"], qd * 512, 512, c.D, ldq) for qd in range(2)]
            for qd in range(4):
                tq = tqp.tile([128, c.DPT * 512], BF16, tag="tq")
                transpose_quarter(xq[qd], tq, ldc)
                if qd + 2 < 4:
                    xq.append(load_rows(io["xw"], (qd + 2) * 512, 512, c.D, ldq))
                lo = qd * 512
                # K^T columns of this quarter
                for m in range(c.HPT):
                    ps = ps_a()
                    for k in range(c.DPT):
                        nc.tensor.matmul(
                            ps[:], wk_t[k][:, ts(m, 128)], tq[:, ts(k, 512)],
                            start=(k == 0), stop=(k == c.DPT - 1))
                    nc.scalar.activation(
                        out=kT[:, m * c.KL + lo: m * c.KL + lo + 512],
                        in_=ps[:], func=AF.Copy)
                # V rows (token-tiles) of this quarter
                for j in range(4):
                    mt = qd * 4 + j
                    ps = ps_b()
                    for k in range(c.DPT):
                        nc.tensor.matmul(
                            ps[:], tq[:, k * 512 + j * 128: k * 512 + (j + 1) * 128],
                            wv_t[k][:, 0:c.HD],
                            start=(k == 0), stop=(k == c.DPT - 1))
                    dst = bass.AP(
                        tensor=vb.tensor, offset=vb.offset + mt * VW,
                        ap=[[c.NTT * VW, 128], [65, c.NHC], [1, c.DH]])
                    nc.vector.tensor_copy(out=dst, in_=ps[:])
                    ones = bass.AP(
                        tensor=vb.tensor, offset=vb.offset + mt * VW + c.DH,
                        ap=[[c.NTT * VW, 128], [65, c.NHC], [1, 1]])
                    nc.vector.memset(ones, 1.0)
                # Q columns (tokens >= M live in quarters 2,3)
                if qd >= 2:
                    qlo = (qd - 2) * 512
                    for m in range(c.HPT):
                        ps = ps_a()
                        for k in range(c.DPT):
                            nc.tensor.matmul(
                                ps[:], wq_t[k][:, ts(m, 128)], tq[:, ts(k, 512)],
                                start=(k == 0), stop=(k == c.DPT - 1))
                        sl = slice(m * c.Q + qlo, m * c.Q + qlo + 512)
                        nc.scalar.activation(out=rwq[:, sl], in_=ps[:],
                                             func=AF.Identity, bias=rwb_s[:, m:m + 1])
                        nc.vector.tensor_scalar_add(out=rrq[:, sl], in0=ps[:],
                                                    scalar1=rrb_s[:, m:m + 1])
            # --- o_proj weights prefetch (consumed in phase D)
            oww_st = load_rows(io["oww"], 0, c.HD, c.D, ldq, tag="ldw", bufs=1)
            for p in range(c.HPT):
                bt = atp.tile([128, c.D], BF16, tag="ow_%d" % p)
                nc.vector.tensor_copy(out=bt[:], in_=oww_st[:, p * c.D:(p + 1) * c.D])
                ow_t.append(bt)

        # ============ phase C: attention ============
        # Per (h, s): BD chunks -> psA -> evac bf16 bdw (DVE/Pool alternate);
        # memset tail -240 (pre-softmax-scale); plain shift-DMA (sync HWDGE)
        # -> sbs; per 512-chunk: ident-matmul(sbs) + AC matmul accumulate in
        # psB; exp(SCALE*x) reads PSUM directly (Act). pb -> pT transpose is
        # issued from the scalar queue (2nd HWDGE ring). AV per (head, half):
        # half 0 needs only jt<12 and runs after s=0..3; half 1 after s=7.
        # Emission is software-pipelined: BD(i+1) before identAC(i) so the
        # in-order PE queue never head-blocks on the shift DMA.
        with tc.tile_pool(name="score", bufs=4) as score, \
             tc.tile_pool(name="scoreT", bufs=2) as scoreT:
            pTs = {}

            def emit_bd(h, s):
                hp, hr = h // 2, (h % 2) * 64
                jc, wr_, wst = c.jcomp(s), c.wreal(s), c.wstart(s)
                bdw = score.tile([128, c.WB], BF16, tag="bdw", bufs=4)
                for i, (lo, hi) in enumerate(chunks(wr_)):
                    ps = ps_a()
                    nc.tensor.matmul(
                        ps[:, 0:hi - lo], _qslice(rrq, c, hp, hr, s),
                        rTp[hr:hr + 64, hp * c.WB + wst + lo: hp * c.WB + wst + hi],
                        start=True, stop=True)
                    if i % 2 == 0:
                        nc.vector.tensor_copy(out=bdw[:, lo:hi],
                                              in_=ps[:, 0:hi - lo])
                    else:
                        nc.scalar.activation(out=bdw[:, lo:hi],
                                             in_=ps[:, 0:hi - lo], func=AF.Copy)
                if jc + 128 > wr_:
                    nc.gpsimd.memset(bdw[:, wr_: jc + 128], -240.0)
                # AC into sb (raw), then shift-accumulate raw BD on top
                sb = score.tile([128, c.KL], BF16, tag="sbs", bufs=4)
                for i, (lo, hi) in enumerate(chunks(jc)):
                    ps = ps_b()
                    nc.tensor.matmul(
                        ps[:, 0:hi - lo], _qslice(rwq, c, hp, hr, s),
                        kT[hr:hr + 64, hp * c.KL + lo: hp * c.KL + hi],
                        start=True, stop=True)
                    if i % 2 == 0:
                        nc.scalar.activation(out=sb[:, lo:hi],
                                             in_=ps[:, 0:hi - lo], func=AF.Copy)
                    else:
                        nc.vector.tensor_copy(out=sb[:, lo:hi],
                                              in_=ps[:, 0:hi - lo])
                diag = bass.AP(tensor=bdw.tensor, offset=bdw.offset + 127,
                               ap=[[c.WB - 1, 128], [1, jc]])
                nc.gpsimd.dma_start(out=sb[:, 0:jc], in_=diag,
                                    accum_op=ALU.add)
                return sb

            def emit_acexp(h, s, sb):
                hp, hr = h // 2, (h % 2) * 64
                jc = c.jcomp(s)
                pb = score.tile([128, c.KL], BF16, tag="pb")
                nc.scalar.activation(out=pb[:, 0:jc], in_=sb[:, 0:jc],
                                     func=AF.Exp, scale=float(c.SCALE))
                pT = pTs[h]
                dstap = bass.AP(
                    tensor=pT.tensor, offset=pT.offset + s * 128,
                    ap=[[c.NJT * c.Q, 128], [c.Q, jc // 128], [1, 128]])
                nc.scalar.dma_start(out=dstap, in_=pb[:, 0:jc], transpose=True)
                if jc < c.KL:
                    z = bass.AP(
                        tensor=pT.tensor,
                        offset=pT.offset + (jc // 128) * c.Q + s * 128,
                        ap=[[c.NJT * c.Q, 128], [c.Q, (c.KL - jc) // 128], [1, 128]])
                    nc.gpsimd.memset(z, 0.0)



            def emit_av(h, half):
                hp, hr = h // 2, (h % 2) * 64
                lo, hi = half * 512, half * 512 + 512
                ps = psV.tile([65, 512], F32, tag="v")
                for jt in range(c.NJT):
                    nc.tensor.matmul(
                        ps[0:65, :], vb[:, jt * VW + h * 65: jt * VW + h * 65 + 65],
                        pTs[h][:, jt * c.Q + lo: jt * c.Q + hi],
                        start=(jt == 0), stop=(jt == c.NJT - 1))
                rd = small.tile([1, 512], F32, tag="rd")
                nc.vector.reciprocal(out=rd[:], in_=ps[64:65, :])
                rdb = small.tile([128, 512], F32, tag="rdb")
                src_b = bass.AP(tensor=rd.tensor, offset=rd.offset,
                                ap=[[512, 1], [0, 64], [1, 512]])
                nc.scalar.dma_start(out=rdb[hr:hr + 64, :], in_=src_b)
                nc.vector.tensor_tensor(
                    out=attnT[hr:hr + 64, hp * c.Q + lo: hp * c.Q + hi],
                    in0=ps[0:64, :], in1=rdb[hr:hr + 64, :], op=ALU.mult)

            def drain(ph, psq, sbs):
                emit_acexp(ph, psq, sbs)
                if psq == 4:
                    emit_av(ph, 0)
                elif psq == 0 and ph > 0:
                    emit_av(ph - 1, 1)

            DEPTH = 3
            items = [(h, s) for h in range(c.NHC) for s in range(c.NS)]
            pend = {}
            for idx, (h, s) in enumerate(items):
                if s == 0:
                    pT = scoreT.tile([128, c.NJT * c.Q], BF16, tag="pT")
                    pTs[h] = pT
                pend[(h, s)] = emit_bd(h, s)
                if idx >= DEPTH:
                    ph, psq = items[idx - DEPTH]
                    drain(ph, psq, pend.pop((ph, psq)))
            for ph, psq in items[-DEPTH:]:
                drain(ph, psq, pend.pop((ph, psq)))
            emit_av(7, 1)

        attk.release()

        # ============ phase D: o_proj -> natural -> ReduceScatter ============
        with tc.tile_pool(name="stageD", bufs=3) as stage:
            for m in range(c.DPT):
                for lo, hi in chunks(c.Q):
                    ps = ps_a()
                    for k in range(c.HPT):
                        nc.tensor.matmul(
                            ps[:, 0:hi - lo], ow_t[k][:, ts(m, 128)],
                            attnT[:, k * c.Q + lo: k * c.Q + hi],
                            start=(k == 0), stop=(k == c.HPT - 1))
                    ob = stage.tile([128, 512], F32, tag="oTs")
                    nc.vector.tensor_copy(out=ob[:, 0:hi - lo], in_=ps[:, 0:hi - lo])
                    for q in range((hi - lo) // 128):
                        pst = psB.tile([128, 128], F32, tag="b")
                        nc.tensor.transpose(pst[:], ob[:, ts(q, 128)], identf[:])
                        onat = stage.tile([128, 128], F32, tag="onat")
                        nc.scalar.activation(out=onat[:], in_=pst[:], func=AF.Copy)
                        nc.sync.dma_start(
                            out=io["rs_bin"][ts(lo // 128 + q, 128), ts(m, 128)],
                            in_=onat[:])
        if collective:
            nc.gpsimd.collective_compute(
                "ReduceScatter", ALU.add, replica_groups=rg,
                ins=[io["rs_bin"].ap().opt()], outs=[io["rs_bout"].ap().opt()])
        else:
            # timeline-sim variant: plain copy standing in for the pair RS
            nc.sync.dma_start(out=io["rs_bout"].ap().opt(),
                              in_=io["rs_bin"].ap()[0:c.TOKF, :].opt())
        atp.release()

        # ============ phase E: LN1 + FFN + LN2 ============
        phE = ctx.enter_context(tc.tile_pool(name="phE", bufs=1))
        eps_t = phE.tile([128, 1], F32, tag="eps")
        nc.vector.memset(eps_t[:], c.LN_EPS)
        lns = {}
        for nm in ("ln1g", "ln1b", "ln2g", "ln2b"):
            tl = phE.tile([128, c.D], F32, tag=nm)
            bcast = bass.AP(tensor=io[nm].ap().tensor, offset=0,
                            ap=[[0, 128], [1, c.D]])
            nc.scalar.dma_start(out=tl[:], in_=bcast)
            lns[nm] = tl
        fb1 = phE.tile([128, c.DI // 128], F32, tag="fb1")
        nc.scalar.dma_start(out=fb1[:], in_=bass.AP(
            tensor=io["ffb1"].ap().tensor, offset=0, ap=[[1, 128], [128, c.DI // 128]]))
        fb2 = phE.tile([128, c.DPT], F32, tag="fb2")
        nc.scalar.dma_start(out=fb2[:], in_=bass.AP(
            tensor=io["ffb2"].ap().tensor, offset=0, ap=[[1, 128], [128, c.DPT]]))

        F32R = mybir.dt.float32r

        with tc.tile_pool(name="ffn", bufs=1) as ffn, \
             tc.tile_pool(name="stageE", bufs=2) as stage, \
             tc.tile_pool(name="wstr", bufs=4) as wstr:
            ntt = c.TOKF // 128
            nkt = c.DI // 128
            ln1n = ffn.tile([128, ntt * c.D], F32, tag="ln1n")
            lnT = ffn.tile([128, c.DPT * c.TOKF], BF16, tag="lnT")
            for tt in range(ntt):
                z = stage.tile([128, c.D], F32, tag="z")
                nc.scalar.dma_start(out=z[:], in_=io["rs_bout"][ts(tt, 128), :])
                wv = stage.tile([128, c.D], F32, tag="wv")
                nc.scalar.dma_start(out=wv[:], in_=io["wres"][ts(tt, 128), :])
                nc.vector.tensor_add(out=z[:], in0=z[:], in1=wv[:])
                _layernorm_nat(nc, c, small, z[:], eps_t,
                               lns["ln1g"], lns["ln1b"],
                               ln1n[:, tt * c.D:(tt + 1) * c.D])
                for p in range(c.DPT):
                    pst = psB.tile([128, 128], F32, tag="b")
                    nc.tensor.transpose(
                        pst[:], ln1n[:, tt * c.D + p * 128: tt * c.D + (p + 1) * 128],
                        identf[:])
                    nc.vector.tensor_copy(
                        out=lnT[:, p * c.TOKF + tt * 128: p * c.TOKF + (tt + 1) * 128],
                        in_=pst[:])
            # FFN1 in bf16 (w1 slices cast on DVE)
            hT = ffn.tile([128, (c.DI // 128) * c.TOKF], BF16, tag="hT")
            for m in range(c.DI // 128):
                w1f = wstr.tile([128, c.DPT * 128], F32, tag="w1f", bufs=3)
                nc.sync.dma_start(out=w1f[:], in_=bass.AP(
                    tensor=io["ffw1"].ap().tensor, offset=m * 128,
                    ap=[[c.DI, 128], [128 * c.DI, c.DPT], [1, 128]]))
                w1m = wstr.tile([128, c.DPT * 128], BF16, tag="w1m", bufs=3)
                nc.vector.tensor_copy(out=w1m[:], in_=w1f[:])
                for lo, hi in chunks(c.TOKF):
                    ps = ps_a()
                    for k in range(c.DPT):
                        nc.tensor.matmul(
                            ps[:, 0:hi - lo],
                            w1m[:, ts(k, 128)],
                            lnT[:, k * c.TOKF + lo: k * c.TOKF + hi],
                            start=(k == 0), stop=(k == c.DPT - 1))
                    nc.scalar.activation(
                        out=hT[:, m * c.TOKF + lo: m * c.TOKF + hi],
                        in_=ps[:, 0:hi - lo], func=AF.Relu, bias=fb1[:, m:m + 1])
            # FFN2 in bf16 (w2 halves cast on DVE, idle in this phase)
            o2T = ffn.tile([128, c.DPT * c.TOKF], F32, tag="o2T")
            for m in range(c.DPT):
                w2m = []
                for hf in range(2):
                    w2f = wstr.tile([128, 16 * 128], F32, tag="w2f", bufs=3)
                    nc.sync.dma_start(out=w2f[:], in_=bass.AP(
                        tensor=io["ffw2"].ap().tensor,
                        offset=hf * 16 * 128 * c.D + m * 128,
                        ap=[[c.D, 128], [128 * c.D, 16], [1, 128]]))
                    w2b = wstr.tile([128, 16 * 128], BF16, tag="w2b", bufs=3)
                    nc.vector.tensor_copy(out=w2b[:], in_=w2f[:])
                    w2m.append(w2b)
                for lo, hi in chunks(c.TOKF):
                    ps = ps_a()
                    for k in range(nkt):
                        nc.tensor.matmul(
                            ps[:, 0:hi - lo],
                            w2m[k // 16][:, (k % 16) * 128:(k % 16 + 1) * 128],
                            hT[:, k * c.TOKF + lo: k * c.TOKF + hi],
                            start=(k == 0), stop=(k == nkt - 1))
                    nc.scalar.activation(
                        out=o2T[:, m * c.TOKF + lo: m * c.TOKF + hi],
                        in_=ps[:, 0:hi - lo], func=AF.Identity, bias=fb2[:, m:m + 1])
            for tt in range(ntt):
                o2n = stage.tile([128, c.D], F32, tag="o2n")
                for p in range(c.DPT):
                    pst = psB.tile([128, 128], F32, tag="b")
                    nc.tensor.transpose(
                        pst[:],
                        o2T[:, p * c.TOKF + tt * 128: p * c.TOKF + (tt + 1) * 128],
                        identf[:])
                    nc.vector.tensor_copy(out=o2n[:, ts(p, 128)], in_=pst[:])
                nc.gpsimd.tensor_tensor(out=o2n[:], in0=o2n[:],
                                        in1=ln1n[:, tt * c.D:(tt + 1) * c.D],
                                        op=ALU.add)
                fin = stage.tile([128, c.D], F32, tag="fin")
                _layernorm_nat(nc, c, small, o2n[:], eps_t,
                               lns["ln2g"], lns["ln2b"], fin[:])
                nc.sync.dma_start(out=io["out"][ts(tt, 128), :], in_=fin[:])


def _layernorm_nat(nc, c, small, z, eps_t, g, b, out_dst):
    """LayerNorm over the free axis of z [128, D] fp32."""
    BN_FMAX = nc.vector.BN_STATS_FMAX
    d = z.shape[-1]
    sub = math.gcd(BN_FMAX, d)
    nsub = d // sub
    zr = z.rearrange("p (n f) -> p n f", f=sub)
    stats = small.tile([128, nsub, nc.vector.BN_STATS_DIM], F32, tag="bnst")
    for i in range(nsub):
        nc.vector.bn_stats(out=stats[:, i, :], in_=zr[:, i, :])
    mv = small.tile([128, nc.vector.BN_AGGR_DIM], F32, tag="bnag")
    nc.vector.bn_aggr(out=mv[:], in_=stats[:])
    mean, var = mv[:, 0:1], mv[:, 1:2]
    nc.scalar.activation(out=var, in_=var, func=AF.Sqrt, bias=eps_t[:], scale=1.0)
    nc.vector.reciprocal(out=var, in_=var)
    nc.vector.tensor_scalar(out=out_dst, in0=z, scalar1=mean, scalar2=var,
                            op0=ALU.subtract, op1=ALU.mult)
    nc.vector.tensor_tensor(out=out_dst, in0=out_dst, in1=g[:, 0:d], op=ALU.mult)
    nc.gpsimd.tensor_tensor(out=out_dst, in0=out_dst, in1=b[:, 0:d], op=ALU.add)


# ============================================================
# host-side sharding + entry point
# ============================================================

def shard_inputs(inputs, c: Cfg = None):
    c = c or Cfg()
    w = np.asarray(inputs["w"], np.float32)
    r = np.asarray(inputs["r"], np.float32)
    mems = np.asarray(inputs["mems"], np.float32)
    qkv_w = np.asarray(inputs["qkv_w"], np.float32)
    r_net_w = np.asarray(inputs["r_net_w"], np.float32)
    o_w = np.asarray(inputs["o_w"], np.float32)
    r_w_bias = np.asarray(inputs["r_w_bias"], np.float32).reshape(-1)
    r_r_bias = np.asarray(inputs["r_r_bias"], np.float32).reshape(-1)
    NHD = qkv_w.shape[1] // 3
    in_maps = []
    for core in range(c.N_CORES):
        b, hh = core // 2, core % 2
        hsl = slice(hh * c.HD, (hh + 1) * c.HD)
        xw_c = np.concatenate([mems[:, b, :], w[:, b, :]], axis=0)
        qkvw_c = np.concatenate([qkv_w[:, j * NHD + hh * c.HD:
                                       j * NHD + (hh + 1) * c.HD]
                                 for j in range(3)], axis=1)
        in_maps.append({
            "xw": np.ascontiguousarray(xw_c),
            "r_in": np.ascontiguousarray(r[:, 0, :]),
            "qkvw": np.ascontiguousarray(qkvw_c),
            "rnetw": np.ascontiguousarray(r_net_w[:, hsl]),
            "oww": np.ascontiguousarray(o_w[hsl, :]),
            "rwb": np.ascontiguousarray(r_w_bias[hsl][None, :]),
            "rrb": np.ascontiguousarray(r_r_bias[hsl][None, :]),
            "ln1g": np.asarray(inputs["ln1g" if "ln1g" in inputs else "ln1_g"],
                               np.float32).reshape(1, -1),
            "ln1b": np.asarray(inputs["ln1b" if "ln1b" in inputs else "ln1_b"],
                               np.float32).reshape(1, -1),
            "ln2g": np.asarray(inputs["ln2g" if "ln2g" in inputs else "ln2_g"],
                               np.float32).reshape(1, -1),
            "ln2b": np.asarray(inputs["ln2b" if "ln2b" in inputs else "ln2_b"],
                               np.float32).reshape(1, -1),
            "ffw1": np.asarray(inputs["ff_w1"], np.float32),
            "ffb1": np.asarray(inputs["ff_b1"], np.float32).reshape(1, -1),
            "ffw2": np.asarray(inputs["ff_w2"], np.float32),
            "ffb2": np.asarray(inputs["ff_b2"], np.float32).reshape(1, -1),
            "wres": np.ascontiguousarray(w[hh * c.TOKF:(hh + 1) * c.TOKF, b, :]),
        })
    return in_maps


def unshard_output(results, inputs, c: Cfg = None):
    c = c or Cfg()
    w = np.asarray(inputs["w"])
    Q, B, D = w.shape
    out = np.zeros((Q, B, D), np.float32)
    for core in range(c.N_CORES):
        b, hh = core // 2, core % 2
        out[hh * c.TOKF:(hh + 1) * c.TOKF, b, :] = results[core]["out"]
    return out


_NC_CACHE = {}


def kernel(**inputs):
    if "nc" not in _NC_CACHE:
        _NC_CACHE["nc"] = build_kernel()
    nc = _NC_CACHE["nc"]
    in_maps = shard_inputs(inputs)
    from concourse.bass_utils import run_bass_kernel_spmd
    res = run_bass_kernel_spmd(nc, in_maps, core_ids=list(range(Cfg.N_CORES)))
    return unshard_output(res.results, inputs)



# revision 3
# speedup vs baseline: 1.1227x; 1.1227x over previous
"""Trainium2 Bass kernel for nn_MemTransformerLM (Transformer-XL layer).

Sharding (8 cores): batch (4) x head-half (2). Every core runs an identical
program over batch b = c//2 and heads [hh*8, hh*8+8), hh = c%2, for all 1024
queries. After o_proj a 2-rank ReduceScatter over core pairs (2b, 2b+1)
splits tokens for the FFN: even core keeps tokens [0,512), odd [512,1024).

Attention rel-shift: BD[i,j] = BD_raw[i, j-i+Q-1] is applied with a
"diagonal" SBUF->SBUF DMA (flat access pattern [[W-1,128],[1,N]]) that
accumulates the shifted BD window into the AC scores. The causal mask is
baked in by memsetting the out-of-range tail of each BD window to -30
before the shift, so exp() zeroes masked lanes without a mask pass.
"""

import contextlib
import math

import numpy as np

import concourse.bass as bass
import concourse.bacc as bacc
import concourse.mybir as mybir
import concourse.tile as tile
from concourse.masks import make_identity

F32 = mybir.dt.float32
BF16 = mybir.dt.bfloat16
AF = mybir.ActivationFunctionType
ALU = mybir.AluOpType


class Cfg:
    D = 1024      # model dim
    NHC = 8       # heads per core
    DH = 64       # head dim
    KL = 2048     # key length
    Q = 1024      # query length
    DI = 4096     # ffn inner
    LN_EPS = 1e-5
    N_CORES = 8

    HD = property(lambda s: s.NHC * s.DH)       # head dims per core
    SCALE = property(lambda s: 1.0 / (s.DH ** 0.5))
    M = property(lambda s: s.KL - s.Q)          # mem length
    NS = property(lambda s: s.Q // 128)         # q tiles
    NJT = property(lambda s: s.KL // 128)       # key tiles
    DPT = property(lambda s: s.D // 128)
    HPT = property(lambda s: s.HD // 128)
    NTT = property(lambda s: s.KL // 128)
    WB = property(lambda s: s.KL + 128)         # BD window buffer width
    TOKF = property(lambda s: s.Q // 2)         # ffn tokens per core

    def jmax(self, s):
        return min(self.KL, 128 * (s + 1) + self.M)

    def jcomp(self, s):
        # 128-aligned causal bound (jmax is already 128-aligned)
        return self.jmax(s)

    def wstart(self, s):
        return self.Q - 128 * (s + 1)

    def wreal(self, s):
        return min(self.jcomp(s) + 128, self.KL - self.wstart(s))


def ts(i, n):
    return slice(i * n, (i + 1) * n)


def chunks(total, sz=512):
    return [(lo, min(total, lo + sz)) for lo in range(0, total, sz)]


def build_kernel(c: Cfg = None, collective=True):
    c = c or Cfg()
    nc = bacc.Bacc("TRN2", target_bir_lowering=False)

    io = {}
    def din(name, shape):
        io[name] = nc.dram_tensor(name, shape, F32, kind="ExternalInput")
    din("xw", [c.KL, c.D])
    din("r_in", [c.KL, c.D])
    din("qkvw", [c.D, 3 * c.HD])
    din("rnetw", [c.D, c.HD])
    din("oww", [c.HD, c.D])
    din("rwb", [1, c.HD])
    din("rrb", [1, c.HD])
    din("ln1g", [1, c.D]); din("ln1b", [1, c.D])
    din("ln2g", [1, c.D]); din("ln2b", [1, c.D])
    din("ffw1", [c.D, c.DI]); din("ffb1", [1, c.DI])
    din("ffw2", [c.DI, c.D]); din("ffb2", [1, c.D])
    din("wres", [c.TOKF, c.D])
    io["out"] = nc.dram_tensor("out", [c.TOKF, c.D], F32, kind="ExternalOutput")
    io["rs_bin"] = nc.dram_tensor("rs_bin", [c.Q, c.D], F32)
    io["rs_bout"] = nc.dram_tensor("rs_bout", [c.TOKF, c.D], F32)

    with tile.TileContext(nc) as tc:
        _body(tc, nc, c, io, collective=collective)
    nc.finalize()
    return nc


def _qslice(buf, c, hp, hr, s):
    """[64,128] lhsT slice for head (hp, hr) and q-tile s of a [128, HPT*Q] buf."""
    return buf[hr:hr + 64, hp * c.Q + s * 128: hp * c.Q + (s + 1) * 128]


def _body(tc, nc, c, io, collective=True):
    ctx = contextlib.ExitStack()
    rg = [[i, i + 1] for i in range(0, c.N_CORES, 2)]
    with ctx:
        small = ctx.enter_context(tc.tile_pool(name="small", bufs=2))
        psA = ctx.enter_context(tc.tile_pool(name="psA", bufs=3, space="PSUM"))
        psB = ctx.enter_context(tc.tile_pool(name="psB", bufs=3, space="PSUM"))
        psV = ctx.enter_context(tc.tile_pool(name="psV", bufs=2, space="PSUM"))

        def ps_a():
            return psA.tile([128, 512], F32, tag="a", name="psa")

        def ps_b():
            return psB.tile([128, 512], F32, tag="b", name="psb")

        keep = ctx.enter_context(tc.tile_pool(name="keep", bufs=1))
        ident = keep.tile([128, 128], BF16, tag="identb")
        make_identity(nc, ident)
        identf = keep.tile([128, 128], F32, tag="identf")
        make_identity(nc, identf)

        # ============ phase A/B: streamed loads + projections ============
        # Loads are issued load-first in quarter batches (one DMA per 512
        # DRAM rows) so transposes never head-block loads on the sync queue;
        # casts chase loads on DVE, transposes chase casts, projection
        # matmuls chase transposed quarters.
        atp = tc.alloc_tile_pool(name="atp", bufs=1)
        attk = tc.alloc_tile_pool(name="attk", bufs=1)
        rTp = attk.tile([128, c.HPT * c.WB], BF16, tag="rTp")
        kT = attk.tile([128, c.HPT * c.KL], BF16, tag="kT")
        VW = c.NHC * 65
        vb = attk.tile([128, c.NTT * VW], BF16, tag="vb")
        rwq = attk.tile([128, c.HPT * c.Q], BF16, tag="rwq")
        rrq = attk.tile([128, c.HPT * c.Q], BF16, tag="rrq")
        attnT = atp.tile([128, c.HPT * c.Q], BF16, tag="attnT")
        ow_t = []

        # biases (scalar queue; tiny)
        rwb_s = keep.tile([128, c.HPT], F32, tag="rwb")
        rrb_s = keep.tile([128, c.HPT], F32, tag="rrb")
        nc.scalar.dma_start(out=rwb_s[:], in_=bass.AP(
            tensor=io["rwb"].ap().tensor, offset=0, ap=[[1, 128], [128, c.HPT]]))
        nc.scalar.dma_start(out=rrb_s[:], in_=bass.AP(
            tensor=io["rrb"].ap().tensor, offset=0, ap=[[1, 128], [128, c.HPT]]))

        def load_rows(src, row0, nrows, width, pool, tag="ldq", bufs=None):
            """DRAM [*, width] rows [row0, row0+nrows) -> [128, n*width]."""
            n = nrows // 128
            t = pool.tile([128, n * width], F32, tag=tag, bufs=bufs)
            ap = bass.AP(tensor=src.ap().tensor, offset=row0 * width,
                         ap=[[width, 128], [128 * width, n], [1, width]])
            nc.scalar.dma_start(out=t[:], in_=ap)
            return t

        def transpose_quarter(qtile, dst, ldc):
            """cast+transpose a [128, 4*D] f32 quarter -> [128, DPT*512] bf16."""
            for j in range(4):
                bft = ldc.tile([128, c.D], BF16, tag="castA")
                nc.vector.tensor_copy(out=bft[:], in_=qtile[:, j * c.D:(j + 1) * c.D])
                dstap = bass.AP(
                    tensor=dst.tensor, offset=dst.offset + j * 128,
                    ap=[[c.DPT * 512, 128], [512, c.DPT], [1, 128]])
                nc.sync.dma_start(out=dstap, in_=bft[:], transpose=True)

        with tc.tile_pool(name="ldq", bufs=2) as ldq, \
             tc.tile_pool(name="tq", bufs=2) as tqp, \
             tc.tile_pool(name="ldc", bufs=3) as ldc, \
             tc.tile_pool(name="wpool", bufs=1) as wpool:
            # --- R: load, transpose, project; quarter-local transposed tiles
            rnw = load_rows(io["rnetw"], 0, c.D, c.HD, ldq, tag="ldw", bufs=1)
            rq = [load_rows(io["r_in"], qd * 512, 512, c.D, ldq) for qd in range(2)]
            wr_t = []
            for p in range(c.DPT):
                bt = wpool.tile([128, c.HD], BF16, tag="wr_%d" % p)
                nc.vector.tensor_copy(out=bt[:], in_=rnw[:, p * c.HD:(p + 1) * c.HD])
                wr_t.append(bt)
            for m in range(c.HPT):
                nc.gpsimd.memset(rTp[:, m * c.WB + c.KL:(m + 1) * c.WB], 0.0)
            for qd in range(4):
                tq = tqp.tile([128, c.DPT * 512], BF16, tag="tq")
                transpose_quarter(rq[qd], tq, ldc)
                if qd + 2 < 4:
                    rq.append(load_rows(io["r_in"], (qd + 2) * 512, 512, c.D, ldq))
                lo = qd * 512
                for m in range(c.HPT):
                    ps = ps_a()
                    for k in range(c.DPT):
                        nc.tensor.matmul(
                            ps[:], wr_t[k][:, ts(m, 128)], tq[:, ts(k, 512)],
                            start=(k == 0), stop=(k == c.DPT - 1))
                    nc.scalar.activation(
                        out=rTp[:, m * c.WB + lo: m * c.WB + lo + 512],
                        in_=ps[:], func=AF.Copy)
            # --- qkv weights (single ldw buffer; halves sequential)
            wq_t, wk_t, wv_t = [], [], []
            for p2 in range(2):
                qkw = load_rows(io["qkvw"], p2 * 512, 512, 3 * c.HD, ldq,
                                tag="ldw", bufs=1)
                for pp in range(4):
                    p = p2 * 4 + pp
                    src = qkw[:, pp * 3 * c.HD:]
                    for lst, j, tag in ((wq_t, 0, "wq"), (wk_t, 1, "wk"), (wv_t, 2, "wv")):
                        bt = wpool.tile([128, c.HD], BF16, tag="%s_%d" % (tag, p))
                        nc.vector.tensor_copy(
                            out=bt[:], in_=src[:, j * c.HD:(j + 1) * c.HD])
                        lst.append(bt)
            # --- X: load, transpose, K/V/Q per quarter
            xq = [load_rows(io["xw"], qd * 512, 512, c.D, ldq) for qd in range(2)]
            for qd in range(4):
                tq = tqp.tile([128, c.DPT * 512], BF16, tag="tq")
                transpose_quarter(xq[qd], tq, ldc)
                if qd + 2 < 4:
                    xq.append(load_rows(io["xw"], (qd + 2) * 512, 512, c.D, ldq))
                lo = qd * 512
                # K^T columns of this quarter
                for m in range(c.HPT):
                    ps = ps_a()
                    for k in range(c.DPT):
                        nc.tensor.matmul(
                            ps[:], wk_t[k][:, ts(m, 128)], tq[:, ts(k, 512)],
                            start=(k == 0), stop=(k == c.DPT - 1))
                    nc.scalar.activation(
                        out=kT[:, m * c.KL + lo: m * c.KL + lo + 512],
                        in_=ps[:], func=AF.Copy)
                # V rows (token-tiles) of this quarter
                for j in range(4):
                    mt = qd * 4 + j
                    ps = ps_b()
                    for k in range(c.DPT):
                        nc.tensor.matmul(
                            ps[:], tq[:, k * 512 + j * 128: k * 512 + (j + 1) * 128],
                            wv_t[k][:, 0:c.HD],
                            start=(k == 0), stop=(k == c.DPT - 1))
                    dst = bass.AP(
                        tensor=vb.tensor, offset=vb.offset + mt * VW,
                        ap=[[c.NTT * VW, 128], [65, c.NHC], [1, c.DH]])
                    nc.vector.tensor_copy(out=dst, in_=ps[:])
                    ones = bass.AP(
                        tensor=vb.tensor, offset=vb.offset + mt * VW + c.DH,
                        ap=[[c.NTT * VW, 128], [65, c.NHC], [1, 1]])
                    nc.vector.memset(ones, 1.0)
                # Q columns (tokens >= M live in quarters 2,3)
                if qd >= 2:
                    qlo = (qd - 2) * 512
                    for m in range(c.HPT):
                        ps = ps_a()
                        for k in range(c.DPT):
                            nc.tensor.matmul(
                                ps[:], wq_t[k][:, ts(m, 128)], tq[:, ts(k, 512)],
                                start=(k == 0), stop=(k == c.DPT - 1))
                        sl = slice(m * c.Q + qlo, m * c.Q + qlo + 512)
                        nc.scalar.activation(out=rwq[:, sl], in_=ps[:],
                                             func=AF.Identity, bias=rwb_s[:, m:m + 1])
                        nc.vector.tensor_scalar_add(out=rrq[:, sl], in0=ps[:],
                                                    scalar1=rrb_s[:, m:m + 1])
            # --- o_proj weights prefetch (consumed in phase D)
            oww_st = load_rows(io["oww"], 0, c.HD, c.D, ldq, tag="ldw", bufs=1)
            for p in range(c.HPT):
                bt = atp.tile([128, c.D], BF16, tag="ow_%d" % p)
                nc.vector.tensor_copy(out=bt[:], in_=oww_st[:, p * c.D:(p + 1) * c.D])
                ow_t.append(bt)

        # ============ phase C: attention ============
        # Per (h, s): BD chunks -> psA -> evac bf16 bdw (DVE/Pool alternate);
        # memset tail -240 (pre-softmax-scale); plain shift-DMA (sync HWDGE)
        # -> sbs; per 512-chunk: ident-matmul(sbs) + AC matmul accumulate in
        # psB; exp(SCALE*x) reads PSUM directly (Act). pb -> pT transpose is
        # issued from the scalar queue (2nd HWDGE ring). AV per (head, half):
        # half 0 needs only jt<12 and runs after s=0..3; half 1 after s=7.
        # Emission is software-pipelined: BD(i+1) before identAC(i) so the
        # in-order PE queue never head-blocks on the shift DMA.
        with tc.tile_pool(name="score", bufs=4) as score, \
             tc.tile_pool(name="scoreT", bufs=2) as scoreT:
            pTs = {}

            def emit_bd(h, s):
                hp, hr = h // 2, (h % 2) * 64
                jc, wr_, wst = c.jcomp(s), c.wreal(s), c.wstart(s)
                bdw = score.tile([128, c.WB], BF16, tag="bdw", bufs=4)
                for i, (lo, hi) in enumerate(chunks(wr_)):
                    ps = ps_a()
                    nc.tensor.matmul(
                        ps[:, 0:hi - lo], _qslice(rrq, c, hp, hr, s),
                        rTp[hr:hr + 64, hp * c.WB + wst + lo: hp * c.WB + wst + hi],
                        start=True, stop=True)
                    # Act carries exp already; give it only every 3rd chunk
                    if i % 3 == 2:
                        nc.scalar.activation(out=bdw[:, lo:hi],
                                             in_=ps[:, 0:hi - lo], func=AF.Copy)
                    else:
                        nc.vector.tensor_copy(out=bdw[:, lo:hi],
                                              in_=ps[:, 0:hi - lo])
                if jc + 128 > wr_:
                    nc.gpsimd.memset(bdw[:, wr_: jc + 128], -240.0)
                # plain shifted-BD band; the AC matmul accumulates on top of
                # an identity-matmul reload, so no AC evacuation is needed
                sbs = score.tile([128, c.KL], BF16, tag="sbs", bufs=4)
                diag = bass.AP(tensor=bdw.tensor, offset=bdw.offset + 127,
                               ap=[[c.WB - 1, 128], [1, jc]])
                nc.sync.dma_start(out=sbs[:, 0:jc], in_=diag)
                return sbs

            def emit_acexp(h, s, sbs):
                hp, hr = h // 2, (h % 2) * 64
                jc = c.jcomp(s)
                pb = score.tile([128, c.KL], BF16, tag="pb")
                for lo, hi in chunks(jc):
                    ps = ps_b()
                    nc.tensor.matmul(ps[:, 0:hi - lo], ident[:],
                                     sbs[:, lo:hi], start=True, stop=False)
                    nc.tensor.matmul(
                        ps[:, 0:hi - lo], _qslice(rwq, c, hp, hr, s),
                        kT[hr:hr + 64, hp * c.KL + lo: hp * c.KL + hi],
                        start=False, stop=True)
                    nc.scalar.activation(out=pb[:, lo:hi], in_=ps[:, 0:hi - lo],
                                         func=AF.Exp, scale=float(c.SCALE))
                pT = pTs[h]
                dstap = bass.AP(
                    tensor=pT.tensor, offset=pT.offset + s * 128,
                    ap=[[c.NJT * c.Q, 128], [c.Q, jc // 128], [1, 128]])
                nc.scalar.dma_start(out=dstap, in_=pb[:, 0:jc], transpose=True)
                if jc < c.KL:
                    z = bass.AP(
                        tensor=pT.tensor,
                        offset=pT.offset + (jc // 128) * c.Q + s * 128,
                        ap=[[c.NJT * c.Q, 128], [c.Q, (c.KL - jc) // 128], [1, 128]])
                    nc.gpsimd.memset(z, 0.0)



            def emit_av(h, half):
                hp, hr = h // 2, (h % 2) * 64
                lo, hi = half * 512, half * 512 + 512
                ps = psV.tile([65, 512], F32, tag="v")
                for jt in range(c.NJT):
                    nc.tensor.matmul(
                        ps[0:65, :], vb[:, jt * VW + h * 65: jt * VW + h * 65 + 65],
                        pTs[h][:, jt * c.Q + lo: jt * c.Q + hi],
                        start=(jt == 0), stop=(jt == c.NJT - 1))
                rd = small.tile([1, 512], F32, tag="rd")
                nc.vector.reciprocal(out=rd[:], in_=ps[64:65, :])
                rdb = small.tile([128, 512], F32, tag="rdb")
                src_b = bass.AP(tensor=rd.tensor, offset=rd.offset,
                                ap=[[512, 1], [0, 64], [1, 512]])
                nc.scalar.dma_start(out=rdb[hr:hr + 64, :], in_=src_b)
                nc.vector.tensor_tensor(
                    out=attnT[hr:hr + 64, hp * c.Q + lo: hp * c.Q + hi],
                    in0=ps[0:64, :], in1=rdb[hr:hr + 64, :], op=ALU.mult)

            def drain(ph, psq, sbs):
                emit_acexp(ph, psq, sbs)
                if psq == 4:
                    emit_av(ph, 0)
                elif psq == 0 and ph > 0:
                    emit_av(ph - 1, 1)

            DEPTH = 3
            items = [(h, s) for h in range(c.NHC) for s in range(c.NS)]
            pend = {}
            for idx, (h, s) in enumerate(items):
                if s == 0:
                    pT = scoreT.tile([128, c.NJT * c.Q], BF16, tag="pT")
                    pTs[h] = pT
                pend[(h, s)] = emit_bd(h, s)
                if idx >= DEPTH:
                    ph, psq = items[idx - DEPTH]
                    drain(ph, psq, pend.pop((ph, psq)))
            for ph, psq in items[-DEPTH:]:
                drain(ph, psq, pend.pop((ph, psq)))
            emit_av(7, 1)

        attk.release()

        # ============ phase D: o_proj -> natural -> ReduceScatter ============
        with tc.tile_pool(name="stageD", bufs=3) as stage:
            for m in range(c.DPT):
                for lo, hi in chunks(c.Q):
                    ps = ps_a()
                    for k in range(c.HPT):
                        nc.tensor.matmul(
                            ps[:, 0:hi - lo], ow_t[k][:, ts(m, 128)],
                            attnT[:, k * c.Q + lo: k * c.Q + hi],
                            start=(k == 0), stop=(k == c.HPT - 1))
                    ob = stage.tile([128, 512], F32, tag="oTs")
                    nc.vector.tensor_copy(out=ob[:, 0:hi - lo], in_=ps[:, 0:hi - lo])
                    for q in range((hi - lo) // 128):
                        pst = psB.tile([128, 128], F32, tag="b")
                        nc.tensor.transpose(pst[:], ob[:, ts(q, 128)], identf[:])
                        onat = stage.tile([128, 128], F32, tag="onat")
                        nc.scalar.activation(out=onat[:], in_=pst[:], func=AF.Copy)
                        nc.sync.dma_start(
                            out=io["rs_bin"][ts(lo // 128 + q, 128), ts(m, 128)],
                            in_=onat[:])
        if collective:
            nc.gpsimd.collective_compute(
                "ReduceScatter", ALU.add, replica_groups=rg,
                ins=[io["rs_bin"].ap().opt()], outs=[io["rs_bout"].ap().opt()])
        else:
            # timeline-sim variant: plain copy standing in for the pair RS
            nc.sync.dma_start(out=io["rs_bout"].ap().opt(),
                              in_=io["rs_bin"].ap()[0:c.TOKF, :].opt())
        atp.release()

        # ============ phase E: LN1 + FFN + LN2 ============
        phE = ctx.enter_context(tc.tile_pool(name="phE", bufs=1))
        eps_t = phE.tile([128, 1], F32, tag="eps")
        nc.vector.memset(eps_t[:], c.LN_EPS)
        lns = {}
        for nm in ("ln1g", "ln1b", "ln2g", "ln2b"):
            tl = phE.tile([128, c.D], F32, tag=nm)
            bcast = bass.AP(tensor=io[nm].ap().tensor, offset=0,
                            ap=[[0, 128], [1, c.D]])
            nc.scalar.dma_start(out=tl[:], in_=bcast)
            lns[nm] = tl
        fb1 = phE.tile([128, c.DI // 128], F32, tag="fb1")
        nc.scalar.dma_start(out=fb1[:], in_=bass.AP(
            tensor=io["ffb1"].ap().tensor, offset=0, ap=[[1, 128], [128, c.DI // 128]]))
        fb2 = phE.tile([128, c.DPT], F32, tag="fb2")
        nc.scalar.dma_start(out=fb2[:], in_=bass.AP(
            tensor=io["ffb2"].ap().tensor, offset=0, ap=[[1, 128], [128, c.DPT]]))

        F32R = mybir.dt.float32r

        with tc.tile_pool(name="ffn", bufs=1) as ffn, \
             tc.tile_pool(name="stageE", bufs=2) as stage, \
             tc.tile_pool(name="wstr", bufs=4) as wstr:
            ntt = c.TOKF // 128
            nkt = c.DI // 128
            ln1n = ffn.tile([128, ntt * c.D], F32, tag="ln1n")
            lnT = ffn.tile([128, c.DPT * c.TOKF], BF16, tag="lnT")
            for tt in range(ntt):
                z = stage.tile([128, c.D], F32, tag="z")
                nc.scalar.dma_start(out=z[:], in_=io["rs_bout"][ts(tt, 128), :])
                wv = stage.tile([128, c.D], F32, tag="wv")
                nc.scalar.dma_start(out=wv[:], in_=io["wres"][ts(tt, 128), :])
                nc.vector.tensor_add(out=z[:], in0=z[:], in1=wv[:])
                _layernorm_nat(nc, c, small, z[:], eps_t,
                               lns["ln1g"], lns["ln1b"],
                               ln1n[:, tt * c.D:(tt + 1) * c.D])
                for p in range(c.DPT):
                    pst = psB.tile([128, 128], F32, tag="b")
                    nc.tensor.transpose(
                        pst[:], ln1n[:, tt * c.D + p * 128: tt * c.D + (p + 1) * 128],
                        identf[:])
                    nc.vector.tensor_copy(
                        out=lnT[:, p * c.TOKF + tt * 128: p * c.TOKF + (tt + 1) * 128],
                        in_=pst[:])
            # FFN1 in bf16 (w1 slices cast on DVE)
            hT = ffn.tile([128, (c.DI // 128) * c.TOKF], BF16, tag="hT")
            for m in range(c.DI // 128):
                w1f = wstr.tile([128, c.DPT * 128], F32, tag="w1f", bufs=3)
                nc.sync.dma_start(out=w1f[:], in_=bass.AP(
                    tensor=io["ffw1"].ap().tensor, offset=m * 128,
                    ap=[[c.DI, 128], [128 * c.DI, c.DPT], [1, 128]]))
                w1m = wstr.tile([128, c.DPT * 128], BF16, tag="w1m", bufs=3)
                nc.vector.tensor_copy(out=w1m[:], in_=w1f[:])
                for lo, hi in chunks(c.TOKF):
                    ps = ps_a()
                    for k in range(c.DPT):
                        nc.tensor.matmul(
                            ps[:, 0:hi - lo],
                            w1m[:, ts(k, 128)],
                            lnT[:, k * c.TOKF + lo: k * c.TOKF + hi],
                            start=(k == 0), stop=(k == c.DPT - 1))
                    nc.scalar.activation(
                        out=hT[:, m * c.TOKF + lo: m * c.TOKF + hi],
                        in_=ps[:, 0:hi - lo], func=AF.Relu, bias=fb1[:, m:m + 1])
            # FFN2 in bf16 (w2 halves cast on DVE, idle in this phase)
            o2T = ffn.tile([128, c.DPT * c.TOKF], F32, tag="o2T")
            for m in range(c.DPT):
                w2m = []
                for hf in range(2):
                    w2f = wstr.tile([128, 16 * 128], F32, tag="w2f", bufs=3)
                    nc.sync.dma_start(out=w2f[:], in_=bass.AP(
                        tensor=io["ffw2"].ap().tensor,
                        offset=hf * 16 * 128 * c.D + m * 128,
                        ap=[[c.D, 128], [128 * c.D, 16], [1, 128]]))
                    w2b = wstr.tile([128, 16 * 128], BF16, tag="w2b", bufs=3)
                    nc.vector.tensor_copy(out=w2b[:], in_=w2f[:])
                    w2m.append(w2b)
                for lo, hi in chunks(c.TOKF):
                    ps = ps_a()
                    for k in range(nkt):
                        nc.tensor.matmul(
                            ps[:, 0:hi - lo],
                            w2m[k // 16][:, (k % 16) * 128:(k % 16 + 1) * 128],
                            hT[:, k * c.TOKF + lo: k * c.TOKF + hi],
                            start=(k == 0), stop=(k == nkt - 1))
                    nc.scalar.activation(
                        out=o2T[:, m * c.TOKF + lo: m * c.TOKF + hi],
                        in_=ps[:, 0:hi - lo], func=AF.Identity, bias=fb2[:, m:m + 1])
            for tt in range(ntt):
                o2n = stage.tile([128, c.D], F32, tag="o2n")
                for p in range(c.DPT):
                    pst = psB.tile([128, 128], F32, tag="b")
                    nc.tensor.transpose(
                        pst[:],
                        o2T[:, p * c.TOKF + tt * 128: p * c.TOKF + (tt + 1) * 128],
                        identf[:])
                    nc.vector.tensor_copy(out=o2n[:, ts(p, 128)], in_=pst[:])
                nc.gpsimd.tensor_tensor(out=o2n[:], in0=o2n[:],
                                        in1=ln1n[:, tt * c.D:(tt + 1) * c.D],
                                        op=ALU.add)
                fin = stage.tile([128, c.D], F32, tag="fin")
                _layernorm_nat(nc, c, small, o2n[:], eps_t,
                               lns["ln2g"], lns["ln2b"], fin[:])
                nc.sync.dma_start(out=io["out"][ts(tt, 128), :], in_=fin[:])


def _layernorm_nat(nc, c, small, z, eps_t, g, b, out_dst):
    """LayerNorm over the free axis of z [128, D] fp32."""
    BN_FMAX = nc.vector.BN_STATS_FMAX
    d = z.shape[-1]
    sub = math.gcd(BN_FMAX, d)
    nsub = d // sub
    zr = z.rearrange("p (n f) -> p n f", f=sub)
    stats = small.tile([128, nsub, nc.vector.BN_STATS_DIM], F32, tag="bnst")
    for i in range(nsub):
        nc.vector.bn_stats(out=stats[:, i, :], in_=zr[:, i, :])
    mv = small.tile([128, nc.vector.BN_AGGR_DIM], F32, tag="bnag")
    nc.vector.bn_aggr(out=mv[:], in_=stats[:])
    mean, var = mv[:, 0:1], mv[:, 1:2]
    nc.scalar.activation(out=var, in_=var, func=AF.Sqrt, bias=eps_t[:], scale=1.0)
    nc.vector.reciprocal(out=var, in_=var)
    nc.vector.tensor_scalar(out=out_dst, in0=z, scalar1=mean, scalar2=var,
                            op0=ALU.subtract, op1=ALU.mult)
    nc.vector.tensor_tensor(out=out_dst, in0=out_dst, in1=g[:, 0:d], op=ALU.mult)
    nc.gpsimd.tensor_tensor(out=out_dst, in0=out_dst, in1=b[:, 0:d], op=ALU.add)


# ============================================================
# host-side sharding + entry point
# ============================================================

def shard_inputs(inputs, c: Cfg = None):
    c = c or Cfg()
    w = np.asarray(inputs["w"], np.float32)
    r = np.asarray(inputs["r"], np.float32)
    mems = np.asarray(inputs["mems"], np.float32)
    qkv_w = np.asarray(inputs["qkv_w"], np.float32)
    r_net_w = np.asarray(inputs["r_net_w"], np.float32)
    o_w = np.asarray(inputs["o_w"], np.float32)
    r_w_bias = np.asarray(inputs["r_w_bias"], np.float32).reshape(-1)
    r_r_bias = np.asarray(inputs["r_r_bias"], np.float32).reshape(-1)
    NHD = qkv_w.shape[1] // 3
    in_maps = []
    for core in range(c.N_CORES):
        b, hh = core // 2, core % 2
        hsl = slice(hh * c.HD, (hh + 1) * c.HD)
        xw_c = np.concatenate([mems[:, b, :], w[:, b, :]], axis=0)
        qkvw_c = np.concatenate([qkv_w[:, j * NHD + hh * c.HD:
                                       j * NHD + (hh + 1) * c.HD]
                                 for j in range(3)], axis=1)
        in_maps.append({
            "xw": np.ascontiguousarray(xw_c),
            "r_in": np.ascontiguousarray(r[:, 0, :]),
            "qkvw": np.ascontiguousarray(qkvw_c),
            "rnetw": np.ascontiguousarray(r_net_w[:, hsl]),
            "oww": np.ascontiguousarray(o_w[hsl, :]),
            "rwb": np.ascontiguousarray(r_w_bias[hsl][None, :]),
            "rrb": np.ascontiguousarray(r_r_bias[hsl][None, :]),
            "ln1g": np.asarray(inputs["ln1g" if "ln1g" in inputs else "ln1_g"],
                               np.float32).reshape(1, -1),
            "ln1b": np.asarray(inputs["ln1b" if "ln1b" in inputs else "ln1_b"],
                               np.float32).reshape(1, -1),
            "ln2g": np.asarray(inputs["ln2g" if "ln2g" in inputs else "ln2_g"],
                               np.float32).reshape(1, -1),
            "ln2b": np.asarray(inputs["ln2b" if "ln2b" in inputs else "ln2_b"],
                               np.float32).reshape(1, -1),
            "ffw1": np.asarray(inputs["ff_w1"], np.float32),
            "ffb1": np.asarray(inputs["ff_b1"], np.float32).reshape(1, -1),
            "ffw2": np.asarray(inputs["ff_w2"], np.float32),
            "ffb2": np.asarray(inputs["ff_b2"], np.float32).reshape(1, -1),
            "wres": np.ascontiguousarray(w[hh * c.TOKF:(hh + 1) * c.TOKF, b, :]),
        })
    return in_maps


def unshard_output(results, inputs, c: Cfg = None):
    c = c or Cfg()
    w = np.asarray(inputs["w"])
    Q, B, D = w.shape
    out = np.zeros((Q, B, D), np.float32)
    for core in range(c.N_CORES):
        b, hh = core // 2, core % 2
        out[hh * c.TOKF:(hh + 1) * c.TOKF, b, :] = results[core]["out"]
    return out


_NC_CACHE = {}


def kernel(**inputs):
    if "nc" not in _NC_CACHE:
        _NC_CACHE["nc"] = build_kernel()
    nc = _NC_CACHE["nc"]
    in_maps = shard_inputs(inputs)
    from concourse.bass_utils import run_bass_kernel_spmd
    res = run_bass_kernel_spmd(nc, in_maps, core_ids=list(range(Cfg.N_CORES)))
    return unshard_output(res.results, inputs)



# revision 4
# speedup vs baseline: 1.1719x; 1.0438x over previous
"""Trainium2 Bass kernel for nn_MemTransformerLM (Transformer-XL layer).

Sharding (8 cores): batch (4) x head-half (2). Every core runs an identical
program over batch b = c//2 and heads [hh*8, hh*8+8), hh = c%2, for all 1024
queries. After o_proj a 2-rank ReduceScatter over core pairs (2b, 2b+1)
splits tokens for the FFN: even core keeps tokens [0,512), odd [512,1024).

Attention rel-shift: BD[i,j] = BD_raw[i, j-i+Q-1] is applied with a
"diagonal" SBUF->SBUF DMA (flat access pattern [[W-1,128],[1,N]]) that
accumulates the shifted BD window into the AC scores. The causal mask is
baked in by memsetting the out-of-range tail of each BD window to -30
before the shift, so exp() zeroes masked lanes without a mask pass.
"""

import contextlib
import math

import numpy as np

import concourse.bass as bass
import concourse.bacc as bacc
import concourse.mybir as mybir
import concourse.tile as tile
from concourse.masks import make_identity

F32 = mybir.dt.float32
BF16 = mybir.dt.bfloat16
AF = mybir.ActivationFunctionType
ALU = mybir.AluOpType


class Cfg:
    D = 1024      # model dim
    NHC = 8       # heads per core
    DH = 64       # head dim
    KL = 2048     # key length
    Q = 1024      # query length
    DI = 4096     # ffn inner
    LN_EPS = 1e-5
    N_CORES = 8

    HD = property(lambda s: s.NHC * s.DH)       # head dims per core
    SCALE = property(lambda s: 1.0 / (s.DH ** 0.5))
    M = property(lambda s: s.KL - s.Q)          # mem length
    NS = property(lambda s: s.Q // 128)         # q tiles
    NJT = property(lambda s: s.KL // 128)       # key tiles
    DPT = property(lambda s: s.D // 128)
    HPT = property(lambda s: s.HD // 128)
    NTT = property(lambda s: s.KL // 128)
    WB = property(lambda s: s.KL + 128)         # BD window buffer width
    TOKF = property(lambda s: s.Q // 2)         # ffn tokens per core

    def jmax(self, s):
        return min(self.KL, 128 * (s + 1) + self.M)

    def jcomp(self, s):
        # 128-aligned causal bound (jmax is already 128-aligned)
        return self.jmax(s)

    def wstart(self, s):
        return self.Q - 128 * (s + 1)

    def wreal(self, s):
        return min(self.jcomp(s) + 128, self.KL - self.wstart(s))


def ts(i, n):
    return slice(i * n, (i + 1) * n)


def chunks(total, sz=512):
    return [(lo, min(total, lo + sz)) for lo in range(0, total, sz)]


def build_kernel(c: Cfg = None, collective=True):
    c = c or Cfg()
    nc = bacc.Bacc("TRN2", target_bir_lowering=False)

    io = {}
    def din(name, shape):
        io[name] = nc.dram_tensor(name, shape, F32, kind="ExternalInput")
    din("xw", [c.KL, c.D])
    din("r_in", [c.KL, c.D])
    din("qkvw", [c.D, 3 * c.HD])
    din("rnetw", [c.D, c.HD])
    din("oww", [c.HD, c.D])
    din("rwb", [1, c.HD])
    din("rrb", [1, c.HD])
    din("ln1g", [1, c.D]); din("ln1b", [1, c.D])
    din("ln2g", [1, c.D]); din("ln2b", [1, c.D])
    din("ffw1", [c.D, c.DI]); din("ffb1", [1, c.DI])
    din("ffw2", [c.DI, c.D]); din("ffb2", [1, c.D])
    din("wres", [c.TOKF, c.D])
    io["out"] = nc.dram_tensor("out", [c.TOKF, c.D], F32, kind="ExternalOutput")
    io["rs_bin"] = nc.dram_tensor("rs_bin", [c.Q, c.D], F32)
    io["rs_bout"] = nc.dram_tensor("rs_bout", [c.TOKF, c.D], F32)

    with tile.TileContext(nc) as tc:
        _body(tc, nc, c, io, collective=collective)
    nc.finalize()
    return nc


def _qslice(buf, c, hp, hr, s):
    """[64,128] lhsT slice for head (hp, hr) and q-tile s of a [128, HPT*Q] buf."""
    return buf[hr:hr + 64, hp * c.Q + s * 128: hp * c.Q + (s + 1) * 128]


def _body(tc, nc, c, io, collective=True):
    ctx = contextlib.ExitStack()
    rg = [[i, i + 1] for i in range(0, c.N_CORES, 2)]
    with ctx:
        small = ctx.enter_context(tc.tile_pool(name="small", bufs=2))
        psA = ctx.enter_context(tc.tile_pool(name="psA", bufs=3, space="PSUM"))
        psB = ctx.enter_context(tc.tile_pool(name="psB", bufs=3, space="PSUM"))
        psV = ctx.enter_context(tc.tile_pool(name="psV", bufs=2, space="PSUM"))

        def ps_a():
            return psA.tile([128, 512], F32, tag="a", name="psa")

        def ps_b():
            return psB.tile([128, 512], F32, tag="b", name="psb")

        keep = ctx.enter_context(tc.tile_pool(name="keep", bufs=1))
        ident = keep.tile([128, 128], BF16, tag="identb")
        make_identity(nc, ident)
        identf = keep.tile([128, 128], F32, tag="identf")
        make_identity(nc, identf)

        # ============ phase A/B: streamed loads + projections ============
        # Loads are issued load-first in quarter batches (one DMA per 512
        # DRAM rows) so transposes never head-block loads on the sync queue;
        # casts chase loads on DVE, transposes chase casts, projection
        # matmuls chase transposed quarters.
        atp = tc.alloc_tile_pool(name="atp", bufs=1)
        attk = tc.alloc_tile_pool(name="attk", bufs=1)
        rTp = attk.tile([128, c.HPT * c.WB], BF16, tag="rTp")
        kT = attk.tile([128, c.HPT * c.KL], BF16, tag="kT")
        VW = c.NHC * 65
        vb = attk.tile([128, c.NTT * VW], BF16, tag="vb")
        rwq = attk.tile([128, c.HPT * c.Q], BF16, tag="rwq")
        rrq = attk.tile([128, c.HPT * c.Q], BF16, tag="rrq")
        attnT = atp.tile([128, c.HPT * c.Q], BF16, tag="attnT")
        ow_t = []

        # biases (scalar queue; tiny)
        rwb_s = keep.tile([128, c.HPT], F32, tag="rwb")
        rrb_s = keep.tile([128, c.HPT], F32, tag="rrb")
        nc.scalar.dma_start(out=rwb_s[:], in_=bass.AP(
            tensor=io["rwb"].ap().tensor, offset=0, ap=[[1, 128], [128, c.HPT]]))
        nc.scalar.dma_start(out=rrb_s[:], in_=bass.AP(
            tensor=io["rrb"].ap().tensor, offset=0, ap=[[1, 128], [128, c.HPT]]))

        def load_rows(src, row0, nrows, width, pool, tag="ldq", bufs=None):
            """DRAM [*, width] rows [row0, row0+nrows) -> [128, n*width]."""
            n = nrows // 128
            t = pool.tile([128, n * width], F32, tag=tag, bufs=bufs)
            ap = bass.AP(tensor=src.ap().tensor, offset=row0 * width,
                         ap=[[width, 128], [128 * width, n], [1, width]])
            nc.scalar.dma_start(out=t[:], in_=ap)
            return t

        def transpose_quarter(qtile, dst, ldc):
            """cast+transpose a [128, 4*D] f32 quarter -> [128, DPT*512] bf16."""
            for j in range(4):
                bft = ldc.tile([128, c.D], BF16, tag="castA")
                nc.vector.tensor_copy(out=bft[:], in_=qtile[:, j * c.D:(j + 1) * c.D])
                dstap = bass.AP(
                    tensor=dst.tensor, offset=dst.offset + j * 128,
                    ap=[[c.DPT * 512, 128], [512, c.DPT], [1, 128]])
                nc.sync.dma_start(out=dstap, in_=bft[:], transpose=True)

        with tc.tile_pool(name="ldq", bufs=2) as ldq, \
             tc.tile_pool(name="tq", bufs=2) as tqp, \
             tc.tile_pool(name="ldc", bufs=3) as ldc, \
             tc.tile_pool(name="wpool", bufs=1) as wpool:
            # --- R: load, transpose, project; quarter-local transposed tiles
            rnw = load_rows(io["rnetw"], 0, c.D, c.HD, ldq, tag="ldw", bufs=1)
            rq = [load_rows(io["r_in"], qd * 512, 512, c.D, ldq) for qd in range(2)]
            wr_t = []
            for p in range(c.DPT):
                bt = wpool.tile([128, c.HD], BF16, tag="wr_%d" % p)
                nc.vector.tensor_copy(out=bt[:], in_=rnw[:, p * c.HD:(p + 1) * c.HD])
                wr_t.append(bt)
            for m in range(c.HPT):
                nc.gpsimd.memset(rTp[:, m * c.WB + c.KL:(m + 1) * c.WB], 0.0)
            for qd in range(4):
                tq = tqp.tile([128, c.DPT * 512], BF16, tag="tq")
                transpose_quarter(rq[qd], tq, ldc)
                if qd + 2 < 4:
                    rq.append(load_rows(io["r_in"], (qd + 2) * 512, 512, c.D, ldq))
                lo = qd * 512
                for m in range(c.HPT):
                    ps = ps_a()
                    for k in range(c.DPT):
                        nc.tensor.matmul(
                            ps[:], wr_t[k][:, ts(m, 128)], tq[:, ts(k, 512)],
                            start=(k == 0), stop=(k == c.DPT - 1))
                    nc.scalar.activation(
                        out=rTp[:, m * c.WB + lo: m * c.WB + lo + 512],
                        in_=ps[:], func=AF.Copy)
            # --- qkv weights (single ldw buffer; halves sequential)
            wq_t, wk_t, wv_t = [], [], []
            for p2 in range(2):
                qkw = load_rows(io["qkvw"], p2 * 512, 512, 3 * c.HD, ldq,
                                tag="ldw", bufs=1)
                for pp in range(4):
                    p = p2 * 4 + pp
                    src = qkw[:, pp * 3 * c.HD:]
                    for lst, j, tag in ((wq_t, 0, "wq"), (wk_t, 1, "wk"), (wv_t, 2, "wv")):
                        bt = wpool.tile([128, c.HD], BF16, tag="%s_%d" % (tag, p))
                        nc.vector.tensor_copy(
                            out=bt[:], in_=src[:, j * c.HD:(j + 1) * c.HD])
                        lst.append(bt)
            # --- X: load, transpose, K/V/Q per quarter
            xq = [load_rows(io["xw"], qd * 512, 512, c.D, ldq) for qd in range(2)]
            for qd in range(4):
                tq = tqp.tile([128, c.DPT * 512], BF16, tag="tq")
                transpose_quarter(xq[qd], tq, ldc)
                if qd + 2 < 4:
                    xq.append(load_rows(io["xw"], (qd + 2) * 512, 512, c.D, ldq))
                lo = qd * 512
                # K^T columns of this quarter
                for m in range(c.HPT):
                    ps = ps_a()
                    for k in range(c.DPT):
                        nc.tensor.matmul(
                            ps[:], wk_t[k][:, ts(m, 128)], tq[:, ts(k, 512)],
                            start=(k == 0), stop=(k == c.DPT - 1))
                    nc.scalar.activation(
                        out=kT[:, m * c.KL + lo: m * c.KL + lo + 512],
                        in_=ps[:], func=AF.Copy)
                # V rows (token-tiles) of this quarter
                for j in range(4):
                    mt = qd * 4 + j
                    ps = ps_b()
                    for k in range(c.DPT):
                        nc.tensor.matmul(
                            ps[:], tq[:, k * 512 + j * 128: k * 512 + (j + 1) * 128],
                            wv_t[k][:, 0:c.HD],
                            start=(k == 0), stop=(k == c.DPT - 1))
                    dst = bass.AP(
                        tensor=vb.tensor, offset=vb.offset + mt * VW,
                        ap=[[c.NTT * VW, 128], [65, c.NHC], [1, c.DH]])
                    nc.vector.tensor_copy(out=dst, in_=ps[:])
                    ones = bass.AP(
                        tensor=vb.tensor, offset=vb.offset + mt * VW + c.DH,
                        ap=[[c.NTT * VW, 128], [65, c.NHC], [1, 1]])
                    nc.vector.memset(ones, 1.0)
                # Q columns (tokens >= M live in quarters 2,3)
                if qd >= 2:
                    qlo = (qd - 2) * 512
                    for m in range(c.HPT):
                        ps = ps_a()
                        for k in range(c.DPT):
                            nc.tensor.matmul(
                                ps[:], wq_t[k][:, ts(m, 128)], tq[:, ts(k, 512)],
                                start=(k == 0), stop=(k == c.DPT - 1))
                        sl = slice(m * c.Q + qlo, m * c.Q + qlo + 512)
                        nc.scalar.activation(out=rwq[:, sl], in_=ps[:],
                                             func=AF.Identity, bias=rwb_s[:, m:m + 1])
                        nc.vector.tensor_scalar_add(out=rrq[:, sl], in0=ps[:],
                                                    scalar1=rrb_s[:, m:m + 1])
            # --- o_proj weights prefetch (consumed in phase D)
            oww_st = load_rows(io["oww"], 0, c.HD, c.D, ldq, tag="ldw", bufs=1)
            for p in range(c.HPT):
                bt = atp.tile([128, c.D], BF16, tag="ow_%d" % p)
                nc.vector.tensor_copy(out=bt[:], in_=oww_st[:, p * c.D:(p + 1) * c.D])
                ow_t.append(bt)

        # ============ phase C: attention ============
        # Per (h, s): BD chunks -> psA -> evac bf16 bdw (DVE/Pool alternate);
        # memset tail -240 (pre-softmax-scale); plain shift-DMA (sync HWDGE)
        # -> sbs; per 512-chunk: ident-matmul(sbs) + AC matmul accumulate in
        # psB; exp(SCALE*x) reads PSUM directly (Act). pb -> pT transpose is
        # issued from the scalar queue (2nd HWDGE ring). AV per (head, half):
        # half 0 needs only jt<12 and runs after s=0..3; half 1 after s=7.
        # Emission is software-pipelined: BD(i+1) before identAC(i) so the
        # in-order PE queue never head-blocks on the shift DMA.
        with tc.tile_pool(name="score", bufs=4) as score, \
             tc.tile_pool(name="scoreT", bufs=2) as scoreT:
            pTs = {}

            def emit_bd(h, s):
                hp, hr = h // 2, (h % 2) * 64
                jc, wr_, wst = c.jcomp(s), c.wreal(s), c.wstart(s)
                bdw = score.tile([128, c.WB], BF16, tag="bdw", bufs=4)
                for i, (lo, hi) in enumerate(chunks(wr_)):
                    ps = ps_a()
                    nc.tensor.matmul(
                        ps[:, 0:hi - lo], _qslice(rrq, c, hp, hr, s),
                        rTp[hr:hr + 64, hp * c.WB + wst + lo: hp * c.WB + wst + hi],
                        start=True, stop=True)
                    # Act carries exp already; give it only every 3rd chunk
                    if i % 3 == 2:
                        nc.scalar.activation(out=bdw[:, lo:hi],
                                             in_=ps[:, 0:hi - lo], func=AF.Copy)
                    else:
                        nc.vector.tensor_copy(out=bdw[:, lo:hi],
                                              in_=ps[:, 0:hi - lo])
                if jc + 128 > wr_:
                    nc.gpsimd.memset(bdw[:, wr_: jc + 128], -240.0)
                # plain shifted-BD band; the AC matmul accumulates on top of
                # an identity-matmul reload, so no AC evacuation is needed
                sbs = score.tile([128, c.KL], BF16, tag="sbs", bufs=4)
                diag = bass.AP(tensor=bdw.tensor, offset=bdw.offset + 127,
                               ap=[[c.WB - 1, 128], [1, jc]])
                nc.sync.dma_start(out=sbs[:, 0:jc], in_=diag)
                return sbs

            def emit_acexp(h, s, sbs):
                hp, hr = h // 2, (h % 2) * 64
                jc = c.jcomp(s)
                pb = score.tile([128, c.KL], BF16, tag="pb")
                for lo, hi in chunks(jc):
                    ps = ps_b()
                    nc.tensor.matmul(ps[:, 0:hi - lo], ident[:],
                                     sbs[:, lo:hi], start=True, stop=False)
                    nc.tensor.matmul(
                        ps[:, 0:hi - lo], _qslice(rwq, c, hp, hr, s),
                        kT[hr:hr + 64, hp * c.KL + lo: hp * c.KL + hi],
                        start=False, stop=True)
                    nc.scalar.activation(out=pb[:, lo:hi], in_=ps[:, 0:hi - lo],
                                         func=AF.Exp, scale=float(c.SCALE))
                pT = pTs[h]
                dstap = bass.AP(
                    tensor=pT.tensor, offset=pT.offset + s * 128,
                    ap=[[c.NJT * c.Q, 128], [c.Q, jc // 128], [1, 128]])
                nc.scalar.dma_start(out=dstap, in_=pb[:, 0:jc], transpose=True)
                if jc < c.KL:
                    z = bass.AP(
                        tensor=pT.tensor,
                        offset=pT.offset + (jc // 128) * c.Q + s * 128,
                        ap=[[c.NJT * c.Q, 128], [c.Q, (c.KL - jc) // 128], [1, 128]])
                    nc.gpsimd.memset(z, 0.0)



            def emit_av(h, half):
                hp, hr = h // 2, (h % 2) * 64
                lo, hi = half * 512, half * 512 + 512
                ps = psV.tile([65, 512], F32, tag="v")
                for jt in range(c.NJT):
                    nc.tensor.matmul(
                        ps[0:65, :], vb[:, jt * VW + h * 65: jt * VW + h * 65 + 65],
                        pTs[h][:, jt * c.Q + lo: jt * c.Q + hi],
                        start=(jt == 0), stop=(jt == c.NJT - 1))
                rd = small.tile([1, 512], F32, tag="rd")
                nc.vector.reciprocal(out=rd[:], in_=ps[64:65, :])
                rdb = small.tile([128, 512], F32, tag="rdb")
                src_b = bass.AP(tensor=rd.tensor, offset=rd.offset,
                                ap=[[512, 1], [0, 64], [1, 512]])
                nc.scalar.dma_start(out=rdb[hr:hr + 64, :], in_=src_b)
                nc.vector.tensor_tensor(
                    out=attnT[hr:hr + 64, hp * c.Q + lo: hp * c.Q + hi],
                    in0=ps[0:64, :], in1=rdb[hr:hr + 64, :], op=ALU.mult)

            def drain(ph, psq, sbs):
                emit_acexp(ph, psq, sbs)
                if psq == 4:
                    emit_av(ph, 0)
                elif psq == 0 and ph > 0:
                    emit_av(ph - 1, 1)

            DEPTH = 3
            items = [(h, s) for h in range(c.NHC) for s in range(c.NS)]
            pend = {}
            for idx, (h, s) in enumerate(items):
                if s == 0:
                    pT = scoreT.tile([128, c.NJT * c.Q], BF16, tag="pT")
                    pTs[h] = pT
                pend[(h, s)] = emit_bd(h, s)
                if idx >= DEPTH:
                    ph, psq = items[idx - DEPTH]
                    drain(ph, psq, pend.pop((ph, psq)))
            for ph, psq in items[-DEPTH:]:
                drain(ph, psq, pend.pop((ph, psq)))
            emit_av(7, 1)

        attk.release()

        # ============ phase D: o_proj -> natural -> ReduceScatter ============
        with tc.tile_pool(name="stageD", bufs=3) as stage:
            for m in range(c.DPT):
                for lo, hi in chunks(c.Q):
                    ps = ps_a()
                    for k in range(c.HPT):
                        nc.tensor.matmul(
                            ps[:, 0:hi - lo], ow_t[k][:, ts(m, 128)],
                            attnT[:, k * c.Q + lo: k * c.Q + hi],
                            start=(k == 0), stop=(k == c.HPT - 1))
                    ob = stage.tile([128, 512], F32, tag="oTs")
                    nc.vector.tensor_copy(out=ob[:, 0:hi - lo], in_=ps[:, 0:hi - lo])
                    # transpose 4 q-blocks into one tile, store with a single
                    # strided DMA (one per (m, chunk) instead of four)
                    onat = stage.tile([128, 512], F32, tag="onat")
                    for q in range((hi - lo) // 128):
                        pst = psB.tile([128, 128], F32, tag="b")
                        nc.tensor.transpose(pst[:], ob[:, ts(q, 128)], identf[:])
                        nc.scalar.activation(out=onat[:, ts(q, 128)],
                                             in_=pst[:], func=AF.Copy)
                    dst = bass.AP(
                        tensor=io["rs_bin"].ap().tensor,
                        offset=lo * c.D + m * 128,
                        ap=[[c.D, 128], [128 * c.D, 4], [1, 128]])
                    nc.sync.dma_start(out=dst, in_=onat[:])
        if collective:
            nc.gpsimd.collective_compute(
                "ReduceScatter", ALU.add, replica_groups=rg,
                ins=[io["rs_bin"].ap().opt()], outs=[io["rs_bout"].ap().opt()])
        else:
            # timeline-sim variant: plain copy standing in for the pair RS
            nc.sync.dma_start(out=io["rs_bout"].ap().opt(),
                              in_=io["rs_bin"].ap()[0:c.TOKF, :].opt())
        atp.release()

        # ============ phase E: LN1 + FFN + LN2 ============
        phE = ctx.enter_context(tc.tile_pool(name="phE", bufs=1))
        eps_t = phE.tile([128, 1], F32, tag="eps")
        nc.vector.memset(eps_t[:], c.LN_EPS)
        lns = {}
        for nm in ("ln1g", "ln1b", "ln2g", "ln2b"):
            tl = phE.tile([128, c.D], F32, tag=nm)
            bcast = bass.AP(tensor=io[nm].ap().tensor, offset=0,
                            ap=[[0, 128], [1, c.D]])
            nc.scalar.dma_start(out=tl[:], in_=bcast)
            lns[nm] = tl
        fb1 = phE.tile([128, c.DI // 128], F32, tag="fb1")
        nc.scalar.dma_start(out=fb1[:], in_=bass.AP(
            tensor=io["ffb1"].ap().tensor, offset=0, ap=[[1, 128], [128, c.DI // 128]]))
        fb2 = phE.tile([128, c.DPT], F32, tag="fb2")
        nc.scalar.dma_start(out=fb2[:], in_=bass.AP(
            tensor=io["ffb2"].ap().tensor, offset=0, ap=[[1, 128], [128, c.DPT]]))

        F32R = mybir.dt.float32r

        with tc.tile_pool(name="ffn", bufs=1) as ffn, \
             tc.tile_pool(name="stageE", bufs=2) as stage, \
             tc.tile_pool(name="wstr", bufs=4) as wstr:
            ntt = c.TOKF // 128
            nkt = c.DI // 128
            ln1n = ffn.tile([128, ntt * c.D], F32, tag="ln1n")
            lnT = ffn.tile([128, c.DPT * c.TOKF], BF16, tag="lnT")
            for tt in range(ntt):
                z = stage.tile([128, c.D], F32, tag="z")
                nc.scalar.dma_start(out=z[:], in_=io["rs_bout"][ts(tt, 128), :])
                wv = stage.tile([128, c.D], F32, tag="wv")
                nc.scalar.dma_start(out=wv[:], in_=io["wres"][ts(tt, 128), :])
                nc.vector.tensor_add(out=z[:], in0=z[:], in1=wv[:])
                _layernorm_nat(nc, c, small, z[:], eps_t,
                               lns["ln1g"], lns["ln1b"],
                               ln1n[:, tt * c.D:(tt + 1) * c.D])
                for p in range(c.DPT):
                    pst = psB.tile([128, 128], F32, tag="b")
                    nc.tensor.transpose(
                        pst[:], ln1n[:, tt * c.D + p * 128: tt * c.D + (p + 1) * 128],
                        identf[:])
                    nc.vector.tensor_copy(
                        out=lnT[:, p * c.TOKF + tt * 128: p * c.TOKF + (tt + 1) * 128],
                        in_=pst[:])
            # FFN1 in bf16 (w1 slices cast on DVE)
            hT = ffn.tile([128, (c.DI // 128) * c.TOKF], BF16, tag="hT")
            for m in range(c.DI // 128):
                w1f = wstr.tile([128, c.DPT * 128], F32, tag="w1f", bufs=3)
                nc.sync.dma_start(out=w1f[:], in_=bass.AP(
                    tensor=io["ffw1"].ap().tensor, offset=m * 128,
                    ap=[[c.DI, 128], [128 * c.DI, c.DPT], [1, 128]]))
                w1m = wstr.tile([128, c.DPT * 128], BF16, tag="w1m", bufs=3)
                nc.vector.tensor_copy(out=w1m[:], in_=w1f[:])
                for lo, hi in chunks(c.TOKF):
                    ps = ps_a()
                    for k in range(c.DPT):
                        nc.tensor.matmul(
                            ps[:, 0:hi - lo],
                            w1m[:, ts(k, 128)],
                            lnT[:, k * c.TOKF + lo: k * c.TOKF + hi],
                            start=(k == 0), stop=(k == c.DPT - 1))
                    nc.scalar.activation(
                        out=hT[:, m * c.TOKF + lo: m * c.TOKF + hi],
                        in_=ps[:, 0:hi - lo], func=AF.Relu, bias=fb1[:, m:m + 1])
            # FFN2 in bf16 (w2 halves cast on DVE, idle in this phase)
            o2T = ffn.tile([128, c.DPT * c.TOKF], F32, tag="o2T")
            for m in range(c.DPT):
                w2m = []
                for hf in range(2):
                    w2f = wstr.tile([128, 16 * 128], F32, tag="w2f", bufs=3)
                    nc.sync.dma_start(out=w2f[:], in_=bass.AP(
                        tensor=io["ffw2"].ap().tensor,
                        offset=hf * 16 * 128 * c.D + m * 128,
                        ap=[[c.D, 128], [128 * c.D, 16], [1, 128]]))
                    w2b = wstr.tile([128, 16 * 128], BF16, tag="w2b", bufs=3)
                    nc.vector.tensor_copy(out=w2b[:], in_=w2f[:])
                    w2m.append(w2b)
                for lo, hi in chunks(c.TOKF):
                    ps = ps_a()
                    for k in range(nkt):
                        nc.tensor.matmul(
                            ps[:, 0:hi - lo],
                            w2m[k // 16][:, (k % 16) * 128:(k % 16 + 1) * 128],
                            hT[:, k * c.TOKF + lo: k * c.TOKF + hi],
                            start=(k == 0), stop=(k == nkt - 1))
                    nc.scalar.activation(
                        out=o2T[:, m * c.TOKF + lo: m * c.TOKF + hi],
                        in_=ps[:, 0:hi - lo], func=AF.Identity, bias=fb2[:, m:m + 1])
            for tt in range(ntt):
                o2n = stage.tile([128, c.D], F32, tag="o2n")
                for p in range(c.DPT):
                    pst = psB.tile([128, 128], F32, tag="b")
                    nc.tensor.transpose(
                        pst[:],
                        o2T[:, p * c.TOKF + tt * 128: p * c.TOKF + (tt + 1) * 128],
                        identf[:])
                    nc.vector.tensor_copy(out=o2n[:, ts(p, 128)], in_=pst[:])
                nc.gpsimd.tensor_tensor(out=o2n[:], in0=o2n[:],
                                        in1=ln1n[:, tt * c.D:(tt + 1) * c.D],
                                        op=ALU.add)
                fin = stage.tile([128, c.D], F32, tag="fin")
                _layernorm_nat(nc, c, small, o2n[:], eps_t,
                               lns["ln2g"], lns["ln2b"], fin[:])
                nc.sync.dma_start(out=io["out"][ts(tt, 128), :], in_=fin[:])


def _layernorm_nat(nc, c, small, z, eps_t, g, b, out_dst):
    """LayerNorm over the free axis of z [128, D] fp32."""
    BN_FMAX = nc.vector.BN_STATS_FMAX
    d = z.shape[-1]
    sub = math.gcd(BN_FMAX, d)
    nsub = d // sub
    zr = z.rearrange("p (n f) -> p n f", f=sub)
    stats = small.tile([128, nsub, nc.vector.BN_STATS_DIM], F32, tag="bnst")
    for i in range(nsub):
        nc.vector.bn_stats(out=stats[:, i, :], in_=zr[:, i, :])
    mv = small.tile([128, nc.vector.BN_AGGR_DIM], F32, tag="bnag")
    nc.vector.bn_aggr(out=mv[:], in_=stats[:])
    mean, var = mv[:, 0:1], mv[:, 1:2]
    nc.scalar.activation(out=var, in_=var, func=AF.Sqrt, bias=eps_t[:], scale=1.0)
    nc.vector.reciprocal(out=var, in_=var)
    nc.vector.tensor_scalar(out=out_dst, in0=z, scalar1=mean, scalar2=var,
                            op0=ALU.subtract, op1=ALU.mult)
    nc.vector.tensor_tensor(out=out_dst, in0=out_dst, in1=g[:, 0:d], op=ALU.mult)
    nc.gpsimd.tensor_tensor(out=out_dst, in0=out_dst, in1=b[:, 0:d], op=ALU.add)


# ============================================================
# host-side sharding + entry point
# ============================================================

def shard_inputs(inputs, c: Cfg = None):
    c = c or Cfg()
    w = np.asarray(inputs["w"], np.float32)
    r = np.asarray(inputs["r"], np.float32)
    mems = np.asarray(inputs["mems"], np.float32)
    qkv_w = np.asarray(inputs["qkv_w"], np.float32)
    r_net_w = np.asarray(inputs["r_net_w"], np.float32)
    o_w = np.asarray(inputs["o_w"], np.float32)
    r_w_bias = np.asarray(inputs["r_w_bias"], np.float32).reshape(-1)
    r_r_bias = np.asarray(inputs["r_r_bias"], np.float32).reshape(-1)
    NHD = qkv_w.shape[1] // 3
    in_maps = []
    for core in range(c.N_CORES):
        b, hh = core // 2, core % 2
        hsl = slice(hh * c.HD, (hh + 1) * c.HD)
        xw_c = np.concatenate([mems[:, b, :], w[:, b, :]], axis=0)
        qkvw_c = np.concatenate([qkv_w[:, j * NHD + hh * c.HD:
                                       j * NHD + (hh + 1) * c.HD]
                                 for j in range(3)], axis=1)
        in_maps.append({
            "xw": np.ascontiguousarray(xw_c),
            "r_in": np.ascontiguousarray(r[:, 0, :]),
            "qkvw": np.ascontiguousarray(qkvw_c),
            "rnetw": np.ascontiguousarray(r_net_w[:, hsl]),
            "oww": np.ascontiguousarray(o_w[hsl, :]),
            "rwb": np.ascontiguousarray(r_w_bias[hsl][None, :]),
            "rrb": np.ascontiguousarray(r_r_bias[hsl][None, :]),
            "ln1g": np.asarray(inputs["ln1g" if "ln1g" in inputs else "ln1_g"],
                               np.float32).reshape(1, -1),
            "ln1b": np.asarray(inputs["ln1b" if "ln1b" in inputs else "ln1_b"],
                               np.float32).reshape(1, -1),
            "ln2g": np.asarray(inputs["ln2g" if "ln2g" in inputs else "ln2_g"],
                               np.float32).reshape(1, -1),
            "ln2b": np.asarray(inputs["ln2b" if "ln2b" in inputs else "ln2_b"],
                               np.float32).reshape(1, -1),
            "ffw1": np.asarray(inputs["ff_w1"], np.float32),
            "ffb1": np.asarray(inputs["ff_b1"], np.float32).reshape(1, -1),
            "ffw2": np.asarray(inputs["ff_w2"], np.float32),
            "ffb2": np.asarray(inputs["ff_b2"], np.float32).reshape(1, -1),
            "wres": np.ascontiguousarray(w[hh * c.TOKF:(hh + 1) * c.TOKF, b, :]),
        })
    return in_maps


def unshard_output(results, inputs, c: Cfg = None):
    c = c or Cfg()
    w = np.asarray(inputs["w"])
    Q, B, D = w.shape
    out = np.zeros((Q, B, D), np.float32)
    for core in range(c.N_CORES):
        b, hh = core // 2, core % 2
        out[hh * c.TOKF:(hh + 1) * c.TOKF, b, :] = results[core]["out"]
    return out


_NC_CACHE = {}


def kernel(**inputs):
    if "nc" not in _NC_CACHE:
        _NC_CACHE["nc"] = build_kernel()
    nc = _NC_CACHE["nc"]
    in_maps = shard_inputs(inputs)
    from concourse.bass_utils import run_bass_kernel_spmd
    res = run_bass_kernel_spmd(nc, in_maps, core_ids=list(range(Cfg.N_CORES)))
    return unshard_output(res.results, inputs)

